# revision 32
# baseline (speedup 1.0000x reference)
"""Trainium2 Bass kernel for a 2-layer Chebyshev KAN.

Computation (degree-5 Chebyshev KAN, matching the reference):
    t1  = tanh(x)
    y1  = sum_d T_d(tanh(t1)) @ C1_d.T + t1 @ Wb1.T + b1
    h   = SiLU(LayerNorm(y1))
    out = sum_d T_d(tanh(h)) @ C2_d.T + h @ Wb2.T + b2

Strategy: data-parallel over the batch dim across 8 NeuronCores (2048 rows =
16 partition tiles of 128 per core); weights replicated, pre-transposed +
cast to bf16 on the host and kept resident in SBUF.  The T_0 == 1 term is
folded into an effective bias on the host, leaving 6 [in,out] matmul
matrices per layer (base + d=1..5).

Pipeline: one 128-row tile at a time.  Per tile g the PE runs an "A"
segment (layer-1: 96 N=512 matmuls) and later a "B" segment (layer-2: 48
matmuls).  Segment order  A{0,1} A2 A3 B0 A4 B1 ... A15 B12 B13 B14 B15.
The serial LayerNorm/SiLU chain (scale/shift fused into the SiLU
activation's per-partition scale/bias) and the Chebyshev feature production
are emitted as hooks inside the matmul sweeps so every engine queue stays
fed and the PE never idles.  Features for two adjacent 128-col i-blocks are
produced per "fill" (PE bf16 transposes at 1 cyc/row + ACT tanh/square +
DVE recurrence on [128,256] slices for the 2x/4x DVE perf modes).  Weights
arrive as one 1.5MB DMA per i-block (i-block-major host layout) on the
sync queue; x tiles ride the gpsimd queue so they are never stuck behind
weight traffic.  The first sweep covers two tiles so PE consumption of
weight blocks stays just behind the DMA arrival rate.
"""

import math

import numpy as np
import ml_dtypes

import concourse.bass as bass
import concourse.tile as tile
from concourse import bacc, mybir
from concourse.bass_utils import run_bass_kernel_spmd

N_CORES = 8
B, D0, D1, D2 = 16384, 1024, 1024, 512
BC = B // N_CORES            # rows per core
NT = BC // 128               # 16 partition tiles per core
LN_EPS = 1e-5

F32 = mybir.dt.float32
BF16 = mybir.dt.bfloat16
AF = mybir.ActivationFunctionType
ALU = mybir.AluOpType

SQRT2 = math.sqrt(2.0)


def _bcast_row(nc, pool, vec_ap, n, name, dtype=F32):
    """Load a [n] DRAM vector broadcast across all 128 partitions."""
    t = pool.tile([128, n], dtype, name=name)
    src = bass.AP(tensor=vec_ap.tensor, offset=vec_ap.offset,
                  ap=[[0, 128], list(vec_ap.ap[0])])
    nc.gpsimd.dma_start(out=t[:], in_=src)
    return t


def _rsqrt(eng, veps, statp, magic_t):
    """1/sqrt(veps) via bit-trick seed + 2 Newton iterations (DVE; the Pool
    engine does not support these opcodes on the V3 ISA).
    veps: [128, 1] f32 (> 0).  Avoids ACT Sqrt so the whole kernel stays on
    one activation table set."""
    I32 = mybir.dt.int32
    j = statp.tile([128, 1], I32, tag="rsj", name="rsj")
    eng.tensor_scalar(j[:], veps[:].bitcast(I32), 1, None,
                      op0=ALU.arith_shift_right)
    y = statp.tile([128, 1], F32, tag="rsy", name="rsy")
    eng.tensor_tensor(y[:].bitcast(I32), magic_t[:], j[:], op=ALU.subtract)
    s = statp.tile([128, 1], F32, tag="rss", name="rss")
    w = statp.tile([128, 1], F32, tag="rsw", name="rsw")
    for _ in range(2):
        eng.tensor_tensor(s[:], y[:], y[:], op=ALU.mult)
        eng.tensor_tensor(s[:], s[:], veps[:], op=ALU.mult)
        eng.tensor_scalar(w[:], s[:], -0.5, 1.5, op0=ALU.mult, op1=ALU.add)
        eng.tensor_tensor(y[:], y[:], w[:], op=ALU.mult)
    return y


def _kernel_body(tc, out_d, x_d, w1_d, w2_d, b1_d, b2_d, g_d, be_d):
    nc = tc.nc
    trivial = g_d is None
    import contextlib
    ctx = contextlib.ExitStack()
    with ctx:
        consts = ctx.enter_context(tc.tile_pool(name="consts", bufs=1))
        wpool = ctx.enter_context(tc.tile_pool(name="wpool", bufs=1))
        xtp = ctx.enter_context(tc.tile_pool(name="xtp", bufs=24))
        hbp = ctx.enter_context(tc.tile_pool(name="hbp", bufs=3))
        y1p = ctx.enter_context(tc.tile_pool(name="y1p", bufs=1))
        chebp = ctx.enter_context(tc.tile_pool(name="chebp", bufs=7))
        upool = ctx.enter_context(tc.tile_pool(name="upool", bufs=4))
        statp = ctx.enter_context(tc.tile_pool(name="statp", bufs=4))
        opool = ctx.enter_context(tc.tile_pool(name="opool", bufs=2))
        ps_y1 = ctx.enter_context(tc.tile_pool(name="ps_y1", bufs=4, space="PSUM"))
        ps_b = ctx.enter_context(tc.tile_pool(name="ps_b", bufs=2, space="PSUM"))
        ps_tr = ctx.enter_context(tc.tile_pool(name="ps_tr", bufs=2, space="PSUM"))

        ident = consts.tile([128, 128], BF16, name="ident")
        ident_dram = nc.inline_tensor(
            np.eye(128, dtype=np.float32).astype(ml_dtypes.bfloat16),
            name="ident_dram")
        magic_t = consts.tile([128, 1], mybir.dt.int32, name="magic_t")
        nc.vector.memset(magic_t[:], 0x5F3759DF)
        # Trigger the (single) ACT table-set load while the first DMAs are in
        # flight: Silu selects silu_and_others, which also covers Tanh/Square/
        # Copy -- the only ACT functions this kernel uses.
        warm = consts.tile([128, 1], F32, name="warm")
        nc.scalar.activation(warm[:], magic_t[:].bitcast(F32), AF.Silu)
        # Warm the PE HAM clock-gate during the otherwise-dead DMA spool-up
        # window (~6-11us): ~4us of dummy matmuls on memset tiles flips the
        # PE to 2.4GHz before the first real matmul arrives.
        wst = consts.tile([128, 128], BF16, name="wst")
        nc.gpsimd.memset(wst[:], 0)
        wmv = consts.tile([128, 512], BF16, name="wmv")
        nc.gpsimd.memset(wmv[:], 0)
        ps_w = ps_tr.tile([128, 512], F32, tag="tr", name="ps_warm")
        for _ in range(13):
            nc.tensor.matmul(ps_w[:], wst[:], wmv[:], start=True, stop=True)

        # ------------------------------------------------------------------
        # bookkeeping
        xt_tiles = {}     # (i, w) -> feature-major x window ([128, 256] bf16)
        hb_tiles = {}     # g -> bf16 silu(ln(y1)) tile
        y1ps = {}         # g -> [h0 bank, h1 bank]
        y2ps = {}         # g -> bank
        cheb_pre = {}     # (tag, g, p) -> cheb pair tile (prefilled)

        def dma_xt(w, i_list=range(8)):
            # x arrives pre-transposed (feature-major) from the host; window w
            # covers batch tiles {2w, 2w+1}.  gpsimd issue queue keeps these
            # off the (weight-heavy) sync queue.
            for i in i_list:
                t = xtp.tile([128, 256], BF16, tag="xt", name=f"xt_{i}_{w}")
                nc.gpsimd.dma_start(out=t[:],
                                    in_=x_d[i, :, w * 256:(w + 1) * 256])
                xt_tiles[(i, w)] = t

        def fill(src, tag, g, p):
            """Produce the 6 stationary feature blocks for i-blocks {2p,2p+1}
            of tile g:
              cheb[:,0] = feature-major input (base feature),
              cheb[:,1] = tanh(.) = T1, cheb[:,2..5] = T2..T5 (recurrence).
            Layer 1 (src=None): cheb[:,0] = tanh(xt window slice) directly --
            no transpose needed.  Layer 2: PE bf16 transpose of batch-major h
            (1 cyc/row) + DVE PSUM-evacuation copy.  Recurrence on [128,2,128]
            slices for the 2x/4x DVE perf modes; squares on ACT."""
            cheb = chebp.tile([128, 6, 2, 128], BF16, tag="cheb",
                              name=f"cb{tag}{g}_{p}")
            if src is None:
                w, c = g // 2, (g % 2) * 128
                for j in range(2):
                    nc.scalar.activation(
                        cheb[:, 0, j, :], xt_tiles[(2 * p + j, w)][:, c:c + 128],
                        AF.Tanh)
            else:
                for j in range(2):
                    i = 2 * p + j
                    tr = ps_tr.tile([128, 128], BF16, tag="tr",
                                    name=f"tr{tag}{g}_{i}")
                    nc.tensor.transpose(tr[:], src[:, i * 128:(i + 1) * 128],
                                        ident[:])
                    nc.vector.tensor_copy(cheb[:, 0, j, :], tr[:])
            # tanh reads SBUF (for L2, the tr bank is released by the fast
            # DVE copy instead of a lagging ACT op)
            nc.scalar.activation(cheb[:, 1], cheb[:, 0], AF.Tanh)
            T1, T2, T3, T4, T5 = (cheb[:, k] for k in range(1, 6))
            sq = upool.tile([128, 2, 128], BF16, tag="u", name="sq")
            nc.scalar.activation(sq[:], T1, AF.Square, scale=SQRT2)  # 2*T1^2
            nc.vector.tensor_scalar(T2, sq[:], 1.0, None, op0=ALU.subtract)
            a = upool.tile([128, 2, 128], BF16, tag="u", name="a")
            nc.vector.tensor_scalar(a[:], T2, 2.0, 1.0, op0=ALU.mult,
                                    op1=ALU.subtract)                # 2*T2-1
            nc.vector.tensor_tensor(T3, T1, a[:], op=ALU.mult)
            sq2 = upool.tile([128, 2, 128], BF16, tag="u", name="sq2")
            nc.scalar.activation(sq2[:], T2, AF.Square, scale=SQRT2)  # 2*T2^2
            nc.vector.tensor_scalar(T4, sq2[:], 1.0, None, op0=ALU.subtract)
            c = upool.tile([128, 2, 128], BF16, tag="u", name="c")
            nc.vector.tensor_tensor(c[:], T2, T3, op=ALU.mult)
            d = upool.tile([128, 2, 128], BF16, tag="u", name="d")
            nc.vector.tensor_scalar(d[:], c[:], 2.0, None, op0=ALU.mult)
            nc.vector.tensor_tensor(T5, d[:], T1, op=ALU.subtract)
            return cheb

        ln_state = {}

        def emit_LN_a(g):
            """bias add (frees the y1 PSUM banks) + bn_stats for tile g."""
            ps = y1ps.pop(g)
            y1 = y1p.tile([128, D1], F32, tag="y1", name=f"y1_{g}")
            for h in range(2):
                sl = slice(h * 512, (h + 1) * 512)
                nc.vector.tensor_add(y1[:, sl], ps[h][:], b1_t[:, sl])
            stats = statp.tile([128, 2, 6], F32, tag="stats", name="stats")
            nc.vector.bn_stats(stats[:, 0, :], y1[:, 0:512])
            nc.vector.bn_stats(stats[:, 1, :], y1[:, 512:1024])
            ln_state[g] = (y1, stats)

        def emit_LN_b(g):
            """aggregate + rsqrt (gpsimd) + fused normalize/SiLU -> hb."""
            y1, stats = ln_state.pop(g)
            mv = statp.tile([128, 2], F32, tag="mv", name="mv")
            nc.vector.bn_aggr(mv[:], stats[:])
            veps = statp.tile([128, 1], F32, tag="veps", name="veps")
            nc.vector.tensor_scalar(veps[:], mv[:, 1:2], LN_EPS, None,
                                    op0=ALU.add)
            rstd = _rsqrt(nc.vector, veps, statp, magic_t)
            hb = hbp.tile([128, D1], BF16, tag="hb", name=f"hb_{g}")
            if trivial:
                # silu((y1 - mu) * rstd) == Silu(y1*rstd + (-mu*rstd)) fused
                # into one ACT op with per-partition scale/bias vectors.
                nmr = statp.tile([128, 1], F32, tag="nmr", name="nmr")
                nc.vector.tensor_scalar(nmr[:], mv[:, 0:1], -1.0, None,
                                        op0=ALU.mult)
                nc.vector.tensor_tensor(nmr[:], nmr[:], rstd[:], op=ALU.mult)
                nc.scalar.activation(hb[:], y1[:], AF.Silu, bias=nmr[:],
                                     scale=rstd[:])
            else:
                nc.vector.tensor_scalar(y1[:], y1[:], mv[:, 0:1], rstd[:],
                                        op0=ALU.subtract, op1=ALU.mult)
                nc.vector.tensor_mul(y1[:], y1[:], g_t[:])
                nc.vector.tensor_add(y1[:], y1[:], be_t[:])
                nc.scalar.activation(hb[:], y1[:], AF.Silu)
            hb_tiles[g] = hb

        def emit_LN(g):
            emit_LN_a(g)
            emit_LN_b(g)

        def emit_evac(k):
            ps = y2ps.pop(k)
            o = opool.tile([128, D2], F32, tag="o", name=f"o_{k}")
            nc.vector.tensor_add(o[:], ps[:], b2_t[:])
            nc.sync.dma_start(out=out_d[k * 128:(k + 1) * 128, :], in_=o[:])

        def run_hooks(hooks, i):
            for fn in hooks.get(i, ()):
                fn()

        def emit_A(g_list, hooks, pad_i0=False):
            """Layer-1 sweep over tiles in g_list.  pad_i0: absorb weight-DMA
            arrival jitter in the very first i-block with dummy matmuls so the
            PE stays busy (and the HAM clock-gate warm) while streaming."""
            for gi, g in enumerate(g_list):
                pool, tg = ((ps_b, "b") if (len(g_list) > 1 and gi == 2)
                            else (ps_y1, "y1"))
                y1ps[g] = [pool.tile([128, 512], F32, tag=tg,
                                     name=f"y1ps_{g}_{h}") for h in range(2)]
            chebs = {g: {0: cheb_pre.pop(("A", g, 0))} for g in g_list}
            for i in range(8):
                run_hooks(hooks, i)
                p = i // 2
                if i % 2 == 0 and p + 1 < 4:
                    for g in g_list:
                        chebs[g][p + 1] = fill(None, "A", g, p + 1)
                for g in g_list:
                    for d in range(6):
                        st = chebs[g][p][:, d, i % 2, :]
                        for h in range(2):
                            nc.tensor.matmul(
                                y1ps[g][h][:], st,
                                w1_sb[:, i, d, h * 512:(h + 1) * 512],
                                start=(i == 0 and d == 0),
                                stop=(i == 7 and d == 5))
                        if pad_i0 and i <= 1 and g == g_list[0]:
                            n_pad = (2 if d < 2 else 1) if i == 0 else (
                                1 if d < 3 else 0)
                            for _ in range(n_pad):
                                nc.tensor.matmul(ps_w[:], wst[:], wmv[:],
                                                 start=True, stop=True)
            run_hooks(hooks, 8)

        def emit_B_split(k, hooks):
            """Final-tile layer-2 sweep: accumulate the two 256-wide output
            halves into separate PSUM banks, h-major, so half 0's bias+DMA
            overlaps half 1's matmuls (separate banks: no PSUM collision)
            and only a 128KB store remains after the last matmul."""
            chebs = {0: cheb_pre.pop(("B", k, 0)), 1: cheb_pre.pop(("B", k, 1))}
            for pn in (2, 3):
                chebs[pn] = fill(hb_tiles[k], "B", k, pn)
            o = opool.tile([128, D2], F32, tag="o", name=f"o_{k}")
            for h in range(2):
                sl = slice(h * 256, (h + 1) * 256)
                y2h = ps_b.tile([128, 256], F32, tag="b", name=f"y2ps_{k}_{h}")
                for i in range(8):
                    if h == 0:
                        run_hooks(hooks, i)
                    for d in range(6):
                        nc.tensor.matmul(y2h[:], chebs[i // 2][:, d, i % 2, :],
                                         w2_sb[:, i, d, sl],
                                         start=(i == 0 and d == 0),
                                         stop=(i == 7 and d == 5))
                nc.vector.tensor_add(o[:, sl], y2h[:], b2_t[:, sl])
                nc.sync.dma_start(out=out_d[k * 128:(k + 1) * 128, sl],
                                  in_=o[:, sl])
            hb_tiles.pop(k)

        def emit_B(k, hooks):
            """Layer-2 sweep for tile k (input hb_tiles[k])."""
            y2 = ps_b.tile([128, 512], F32, tag="b", name=f"y2ps_{k}")
            y2ps[k] = y2
            chebs = {0: cheb_pre.pop(("B", k, 0))}
            for i in range(8):
                run_hooks(hooks, i)
                p = i // 2
                # deeper fill-ahead than the A sweeps: the B sweep consumes an
                # i-block every 1.28us, barely more than the ~2.8us fill chain
                # latency / 2 blocks, so produce two pairs up front.
                for pn in ((1, 2) if i == 0 else (3,) if i == 2 else ()):
                    chebs[pn] = fill(hb_tiles[k], "B", k, pn)
                for d in range(6):
                    nc.tensor.matmul(y2[:], chebs[p][:, d, i % 2, :],
                                     w2_sb[:, i, d, :],
                                     start=(i == 0 and d == 0),
                                     stop=(i == 7 and d == 5))
            hb_tiles.pop(k, None)
            run_hooks(hooks, 8)

        # ------------------------------------------------------------------
        # startup: x tiles (gpsimd queue) + their tanh first, then weights
        # (sync queue, one 1.5MB transfer per i-block, i=0 split per-d so the
        # first matmuls aren't gated on a full block), bias vectors last.
        dma_xt(0, [0, 1])

        w1_sb = wpool.tile([128, 8, 6, D1], BF16, name="w1_sb")
        w2_sb = wpool.tile([128, 8, 6, D2], BF16, name="w2_sb")
        for i in range(3):
            for d in range(6):
                src = w1_d[i, :, d, :]
                if i == 0 and d == 0:
                    for q in range(2):
                        nc.sync.dma_start(
                            out=w1_sb[:, 0, 0, q * 512:(q + 1) * 512],
                            in_=src[:, q * 512:(q + 1) * 512])
                else:
                    nc.sync.dma_start(out=w1_sb[:, i, d, :], in_=src)
            if i == 0:
                dma_xt(0, [2, 3, 4, 5, 6, 7])
                dma_xt(1)
        for i in range(3, 8):
            nc.sync.dma_start(out=w1_sb[:, i], in_=w1_d[i])
        for i in range(8):
            nc.sync.dma_start(out=w2_sb[:, i], in_=w2_d[i])
        # ident is first needed by the first layer-2 prefill transpose (~80us)
        nc.sync.dma_start(out=ident[:], in_=ident_dram.ap())

        b1_t = _bcast_row(nc, consts, b1_d, D1, "b1_t")
        b2_t = _bcast_row(nc, consts, b2_d, D2, "b2_t")
        g_t = (None if trivial else _bcast_row(nc, consts, g_d, D1, "g_t", BF16))
        be_t = (None if trivial else _bcast_row(nc, consts, be_d, D1, "be_t",
                                                BF16))

        for g in range(3):
            cheb_pre[("A", g, 0)] = fill(None, "A", g, 0)

        # ------------------------------------------------------------------
        # segment schedule: S0=A{0,1}, A2, A3, B0, A4, B1, ..., A15, B12..B15
        def setd(key, tag, g, p, src_map):
            cheb_pre[key] = fill(None if src_map is None else src_map[g],
                                 tag, g, p)

        emit_A([0, 1, 2], pad_i0=True, hooks={
            4: [lambda: dma_xt(2)],
            5: [lambda: setd(("A", 3, 0), "A", 3, 0, None)],
            8: [lambda: emit_LN(0)],
        })

        def A_hooks(g):
            h = {}
            add = lambda i, fn: h.setdefault(i, []).append(fn)
            if g == 3:
                add(1, lambda: emit_LN_a(1))
                add(2, lambda: emit_LN_b(1))
                add(3, lambda: setd(("B", 0, 0), "B", 0, 0, hb_tiles))
                add(4, lambda: emit_LN_a(2))
                add(5, lambda: emit_LN_b(2))
            else:
                add(1, lambda: emit_LN_a(g - 1))
                # prefill for B_{g-3} at i==2: ahead of this segment's p2/p3
                # fills and LN_b in the ACT/DVE queues, so its ~2.8us serial
                # chain lands well before the B sweep starts
                add(2, lambda: setd(("B", g - 3, 0), "B", g - 3, 0, hb_tiles))
                add(3, lambda: emit_LN_b(g - 1))
            if g >= 4:
                add(2, lambda: emit_evac(g - 4))
            if g in (3, 5, 7, 9, 11):
                add(4, lambda: dma_xt((g + 3) // 2))
            return h

        def B_hooks(k):
            h = {}
            add = lambda i, fn: h.setdefault(i, []).append(fn)
            if k + 4 <= 15:
                # next segment is A_{k+4}
                add(4, lambda: setd(("A", k + 4, 0), "A", k + 4, 0, None))
            if k == 12:
                add(3, lambda: setd(("B", 13, 0), "B", 13, 0, hb_tiles))
            if k == 13:
                # LN(15) here (not at B12): writing hb(15) recycles hb(12)'s
                # slot, whose readers (B12's fills) must all be emitted first.
                add(1, lambda: emit_LN_a(15))
                add(3, lambda: emit_LN_b(15))
            if k in (13, 14):
                add(4, lambda: setd(("B", k + 1, 0), "B", k + 1, 0, hb_tiles))
            if k == 14:
                add(5, lambda: setd(("B", 15, 1), "B", 15, 1, hb_tiles))
            if k >= 13:
                add(2, lambda: emit_evac(k - 1))
            return h

        emit_A([3], A_hooks(3))
        for k in range(12):
            emit_B(k, B_hooks(k))
            emit_A([k + 4], A_hooks(k + 4))
        for k in range(12, 15):
            emit_B(k, B_hooks(k))
        emit_B_split(15, B_hooks(15))


_PROGRAMS = {}


def _get_program(trivial_affine: bool):
    key = trivial_affine
    if key in _PROGRAMS:
        return _PROGRAMS[key]
    nc = bacc.Bacc("TRN2", target_bir_lowering=False, debug=False,
                   num_devices=N_CORES)
    x_d = nc.dram_tensor("x_in", [8, 128, BC], BF16, kind="ExternalInput").ap()
    w1_d = nc.dram_tensor("w1", [8, 128, 6, D1], BF16, kind="ExternalInput").ap()
    w2_d = nc.dram_tensor("w2", [8, 128, 6, D2], BF16, kind="ExternalInput").ap()
    b1_d = nc.dram_tensor("b1e", [D1], F32, kind="ExternalInput").ap()
    b2_d = nc.dram_tensor("b2e", [D2], F32, kind="ExternalInput").ap()
    if trivial_affine:
        g_d = be_d = None
    else:
        g_d = nc.dram_tensor("gam", [D1], BF16, kind="ExternalInput").ap()
        be_d = nc.dram_tensor("bet", [D1], BF16, kind="ExternalInput").ap()
    out_d = nc.dram_tensor("out", [BC, D2], F32, kind="ExternalOutput").ap()

    with tile.TileContext(nc) as tc:
        _kernel_body(tc, out_d, x_d, w1_d, w2_d, b1_d, b2_d, g_d, be_d)
    nc.compile()
    _PROGRAMS[key] = nc
    return nc


def _prep_inputs(x, coeff1, base_w1, bias1, ln_gamma, ln_beta, coeff2,
                 base_w2, bias2):
    # feature-major (transposed) per-core x: [i_block, feat_in_block, batch]
    x = np.asarray(x, np.float32).astype(ml_dtypes.bfloat16)
    coeff1 = np.asarray(coeff1, np.float32)
    coeff2 = np.asarray(coeff2, np.float32)

    # layout: [i_block, row_in_block, d, out] so each i-block is one
    # contiguous 1.5MB DMA transfer.
    w1 = np.empty((8, 128, 6, D1), ml_dtypes.bfloat16)
    w1[:, :, 0, :] = np.asarray(base_w1, np.float32).T.reshape(8, 128, D1)
    for d in range(1, 6):
        w1[:, :, d, :] = coeff1[:, :, d].T.reshape(8, 128, D1)
    w2 = np.empty((8, 128, 6, D2), ml_dtypes.bfloat16)
    w2[:, :, 0, :] = np.asarray(base_w2, np.float32).T.reshape(8, 128, D2)
    for d in range(1, 6):
        w2[:, :, d, :] = coeff2[:, :, d].T.reshape(8, 128, D2)
    b1e = (np.asarray(bias1, np.float32)
           + coeff1[:, :, 0].sum(axis=1)).astype(np.float32)
    b2e = (np.asarray(bias2, np.float32)
           + coeff2[:, :, 0].sum(axis=1)).astype(np.float32)

    g = np.asarray(ln_gamma, np.float32)
    be = np.asarray(ln_beta, np.float32)
    trivial = bool(np.all(g == 1.0) and np.all(be == 0.0))

    shared = {"w1": w1, "w2": w2, "b1e": b1e, "b2e": b2e}
    if not trivial:
        shared["gam"] = g.astype(ml_dtypes.bfloat16)
        shared["bet"] = be.astype(ml_dtypes.bfloat16)
    in_maps = []
    for cid in range(N_CORES):
        m = dict(shared)
        xc = x[cid * BC:(cid + 1) * BC]
        m["x_in"] = np.ascontiguousarray(xc.T.reshape(8, 128, BC))
        in_maps.append(m)
    return trivial, in_maps


def kernel_run(trace=False, **inputs):
    trivial, in_maps = _prep_inputs(**inputs)
    nc = _get_program(trivial)
    res = run_bass_kernel_spmd(nc, in_maps, core_ids=list(range(N_CORES)),
                               trace=trace)
    out = np.concatenate([r["out"] for r in res.results], axis=0)
    return out, res


def kernel(**inputs):
    out, _ = kernel_run(trace=False, **inputs)
    return out


# revision 34
# speedup vs baseline: 1.0035x; 1.0035x over previous
"""Trainium2 Bass kernel for a 2-layer Chebyshev KAN.

Computation (degree-5 Chebyshev KAN, matching the reference):
    t1  = tanh(x)
    y1  = sum_d T_d(tanh(t1)) @ C1_d.T + t1 @ Wb1.T + b1
    h   = SiLU(LayerNorm(y1))
    out = sum_d T_d(tanh(h)) @ C2_d.T + h @ Wb2.T + b2

Strategy: data-parallel over the batch dim across 8 NeuronCores (2048 rows =
16 partition tiles of 128 per core); weights replicated, pre-transposed +
cast to bf16 on the host and kept resident in SBUF.  The T_0 == 1 term is
folded into an effective bias on the host, leaving 6 [in,out] matmul
matrices per layer (base + d=1..5).

Pipeline: one 128-row tile at a time.  Per tile g the PE runs an "A"
segment (layer-1: 96 N=512 matmuls) and later a "B" segment (layer-2: 48
matmuls).  Segment order  A{0,1} A2 A3 B0 A4 B1 ... A15 B12 B13 B14 B15.
The serial LayerNorm/SiLU chain (scale/shift fused into the SiLU
activation's per-partition scale/bias) and the Chebyshev feature production
are emitted as hooks inside the matmul sweeps so every engine queue stays
fed and the PE never idles.  Features for two adjacent 128-col i-blocks are
produced per "fill" (PE bf16 transposes at 1 cyc/row + ACT tanh/square +
DVE recurrence on [128,256] slices for the 2x/4x DVE perf modes).  Weights
arrive as one 1.5MB DMA per i-block (i-block-major host layout) on the
sync queue; x tiles ride the gpsimd queue so they are never stuck behind
weight traffic.  The first sweep covers two tiles so PE consumption of
weight blocks stays just behind the DMA arrival rate.
"""

import math

import numpy as np
import ml_dtypes

import concourse.bass as bass
import concourse.tile as tile
from concourse import bacc, mybir
from concourse.bass_utils import run_bass_kernel_spmd

N_CORES = 8
B, D0, D1, D2 = 16384, 1024, 1024, 512
BC = B // N_CORES            # rows per core
NT = BC // 128               # 16 partition tiles per core
LN_EPS = 1e-5

F32 = mybir.dt.float32
BF16 = mybir.dt.bfloat16
AF = mybir.ActivationFunctionType
ALU = mybir.AluOpType

SQRT2 = math.sqrt(2.0)


def _bcast_row(nc, pool, vec_ap, n, name, dtype=F32):
    """Load a [n] DRAM vector broadcast across all 128 partitions."""
    t = pool.tile([128, n], dtype, name=name)
    src = bass.AP(tensor=vec_ap.tensor, offset=vec_ap.offset,
                  ap=[[0, 128], list(vec_ap.ap[0])])
    nc.gpsimd.dma_start(out=t[:], in_=src)
    return t


def _rsqrt(eng, veps, statp, magic_t):
    """1/sqrt(veps) via bit-trick seed + 2 Newton iterations (DVE; the Pool
    engine does not support these opcodes on the V3 ISA).
    veps: [128, 1] f32 (> 0).  Avoids ACT Sqrt so the whole kernel stays on
    one activation table set."""
    I32 = mybir.dt.int32
    j = statp.tile([128, 1], I32, tag="rsj", name="rsj")
    eng.tensor_scalar(j[:], veps[:].bitcast(I32), 1, None,
                      op0=ALU.arith_shift_right)
    y = statp.tile([128, 1], F32, tag="rsy", name="rsy")
    eng.tensor_tensor(y[:].bitcast(I32), magic_t[:], j[:], op=ALU.subtract)
    s = statp.tile([128, 1], F32, tag="rss", name="rss")
    w = statp.tile([128, 1], F32, tag="rsw", name="rsw")
    for _ in range(2):
        eng.tensor_tensor(s[:], y[:], y[:], op=ALU.mult)
        eng.tensor_tensor(s[:], s[:], veps[:], op=ALU.mult)
        eng.tensor_scalar(w[:], s[:], -0.5, 1.5, op0=ALU.mult, op1=ALU.add)
        eng.tensor_tensor(y[:], y[:], w[:], op=ALU.mult)
    return y


def _kernel_body(tc, out_d, x_d, w1_d, w2_d, b1_d, b2_d, g_d, be_d):
    nc = tc.nc
    trivial = g_d is None
    import contextlib
    ctx = contextlib.ExitStack()
    with ctx:
        consts = ctx.enter_context(tc.tile_pool(name="consts", bufs=1))
        wpool = ctx.enter_context(tc.tile_pool(name="wpool", bufs=1))
        xtp = ctx.enter_context(tc.tile_pool(name="xtp", bufs=24))
        hbp = ctx.enter_context(tc.tile_pool(name="hbp", bufs=3))
        y1p = ctx.enter_context(tc.tile_pool(name="y1p", bufs=1))
        chebp = ctx.enter_context(tc.tile_pool(name="chebp", bufs=7))
        upool = ctx.enter_context(tc.tile_pool(name="upool", bufs=4))
        statp = ctx.enter_context(tc.tile_pool(name="statp", bufs=4))
        opool = ctx.enter_context(tc.tile_pool(name="opool", bufs=2))
        ps_y1 = ctx.enter_context(tc.tile_pool(name="ps_y1", bufs=4, space="PSUM"))
        ps_b = ctx.enter_context(tc.tile_pool(name="ps_b", bufs=2, space="PSUM"))
        ps_tr = ctx.enter_context(tc.tile_pool(name="ps_tr", bufs=2, space="PSUM"))

        ident = consts.tile([128, 128], BF16, name="ident")
        ident_dram = nc.inline_tensor(
            np.eye(128, dtype=np.float32).astype(ml_dtypes.bfloat16),
            name="ident_dram")
        magic_t = consts.tile([128, 1], mybir.dt.int32, name="magic_t")
        nc.vector.memset(magic_t[:], 0x5F3759DF)
        # Trigger the (single) ACT table-set load while the first DMAs are in
        # flight: Silu selects silu_and_others, which also covers Tanh/Square/
        # Copy -- the only ACT functions this kernel uses.
        warm = consts.tile([128, 1], F32, name="warm")
        nc.scalar.activation(warm[:], magic_t[:].bitcast(F32), AF.Silu)
        # Warm the PE HAM clock-gate during the otherwise-dead DMA spool-up
        # window (~6-11us): ~4us of dummy matmuls on memset tiles flips the
        # PE to 2.4GHz before the first real matmul arrives.
        wst = consts.tile([128, 128], BF16, name="wst")
        nc.gpsimd.memset(wst[:], 0)
        wmv = consts.tile([128, 512], BF16, name="wmv")
        nc.gpsimd.memset(wmv[:], 0)
        ps_w = ps_tr.tile([128, 512], F32, tag="tr", name="ps_warm")
        for _ in range(13):
            nc.tensor.matmul(ps_w[:], wst[:], wmv[:], start=True, stop=True)

        # ------------------------------------------------------------------
        # bookkeeping
        xt_tiles = {}     # (i, w) -> feature-major x window ([128, 256] bf16)
        hb_tiles = {}     # g -> bf16 silu(ln(y1)) tile
        y1ps = {}         # g -> [h0 bank, h1 bank]
        y2ps = {}         # g -> bank
        cheb_pre = {}     # (tag, g, p) -> cheb pair tile (prefilled)

        def dma_xt(w, i_list=range(8)):
            # x arrives pre-transposed (feature-major) from the host; window w
            # covers batch tiles {2w, 2w+1}.  gpsimd issue queue keeps these
            # off the (weight-heavy) sync queue.
            for i in i_list:
                t = xtp.tile([128, 256], BF16, tag="xt", name=f"xt_{i}_{w}")
                nc.gpsimd.dma_start(out=t[:],
                                    in_=x_d[i, :, w * 256:(w + 1) * 256])
                xt_tiles[(i, w)] = t

        def fill(src, tag, g, p):
            """Produce the 6 stationary feature blocks for i-blocks {2p,2p+1}
            of tile g:
              cheb[:,0] = feature-major input (base feature),
              cheb[:,1] = tanh(.) = T1, cheb[:,2..5] = T2..T5 (recurrence).
            Layer 1 (src=None): cheb[:,0] = tanh(xt window slice) directly --
            no transpose needed.  Layer 2: PE bf16 transpose of batch-major h
            (1 cyc/row) + DVE PSUM-evacuation copy.  Recurrence on [128,2,128]
            slices for the 2x/4x DVE perf modes; squares on ACT."""
            cheb = chebp.tile([128, 6, 2, 128], BF16, tag="cheb",
                              name=f"cb{tag}{g}_{p}")
            if src is None:
                w, c = g // 2, (g % 2) * 128
                for j in range(2):
                    nc.scalar.activation(
                        cheb[:, 0, j, :], xt_tiles[(2 * p + j, w)][:, c:c + 128],
                        AF.Tanh)
            else:
                for j in range(2):
                    i = 2 * p + j
                    tr = ps_tr.tile([128, 128], BF16, tag="tr",
                                    name=f"tr{tag}{g}_{i}")
                    nc.tensor.transpose(tr[:], src[:, i * 128:(i + 1) * 128],
                                        ident[:])
                    nc.vector.tensor_copy(cheb[:, 0, j, :], tr[:])
            # tanh reads SBUF (for L2, the tr bank is released by the fast
            # DVE copy instead of a lagging ACT op)
            nc.scalar.activation(cheb[:, 1], cheb[:, 0], AF.Tanh)
            T1, T2, T3, T4, T5 = (cheb[:, k] for k in range(1, 6))
            sq = upool.tile([128, 2, 128], BF16, tag="u", name="sq")
            nc.scalar.activation(sq[:], T1, AF.Square, scale=SQRT2)  # 2*T1^2
            nc.vector.tensor_scalar(T2, sq[:], 1.0, None, op0=ALU.subtract)
            a = upool.tile([128, 2, 128], BF16, tag="u", name="a")
            nc.vector.tensor_scalar(a[:], T2, 2.0, 1.0, op0=ALU.mult,
                                    op1=ALU.subtract)                # 2*T2-1
            nc.vector.tensor_tensor(T3, T1, a[:], op=ALU.mult)
            sq2 = upool.tile([128, 2, 128], BF16, tag="u", name="sq2")
            nc.scalar.activation(sq2[:], T2, AF.Square, scale=SQRT2)  # 2*T2^2
            nc.vector.tensor_scalar(T4, sq2[:], 1.0, None, op0=ALU.subtract)
            c = upool.tile([128, 2, 128], BF16, tag="u", name="c")
            nc.vector.tensor_tensor(c[:], T2, T3, op=ALU.mult)
            d = upool.tile([128, 2, 128], BF16, tag="u", name="d")
            nc.vector.tensor_scalar(d[:], c[:], 2.0, None, op0=ALU.mult)
            nc.vector.tensor_tensor(T5, d[:], T1, op=ALU.subtract)
            return cheb

        ln_state = {}

        def emit_LN_a(g):
            """bias add (frees the y1 PSUM banks) + bn_stats for tile g."""
            ps = y1ps.pop(g)
            y1 = y1p.tile([128, D1], F32, tag="y1", name=f"y1_{g}")
            for h in range(2):
                sl = slice(h * 512, (h + 1) * 512)
                nc.vector.tensor_add(y1[:, sl], ps[h][:], b1_t[:, sl])
            stats = statp.tile([128, 2, 6], F32, tag="stats", name="stats")
            nc.vector.bn_stats(stats[:, 0, :], y1[:, 0:512])
            nc.vector.bn_stats(stats[:, 1, :], y1[:, 512:1024])
            ln_state[g] = (y1, stats)

        def emit_LN_b(g):
            """aggregate + rsqrt (gpsimd) + fused normalize/SiLU -> hb."""
            y1, stats = ln_state.pop(g)
            mv = statp.tile([128, 2], F32, tag="mv", name="mv")
            nc.vector.bn_aggr(mv[:], stats[:])
            veps = statp.tile([128, 1], F32, tag="veps", name="veps")
            nc.vector.tensor_scalar(veps[:], mv[:, 1:2], LN_EPS, None,
                                    op0=ALU.add)
            rstd = _rsqrt(nc.vector, veps, statp, magic_t)
            hb = hbp.tile([128, D1], BF16, tag="hb", name=f"hb_{g}")
            if trivial:
                # silu((y1 - mu) * rstd) == Silu(y1*rstd + (-mu*rstd)) fused
                # into one ACT op with per-partition scale/bias vectors.
                nmr = statp.tile([128, 1], F32, tag="nmr", name="nmr")
                nc.vector.tensor_scalar(nmr[:], mv[:, 0:1], -1.0, None,
                                        op0=ALU.mult)
                nc.vector.tensor_tensor(nmr[:], nmr[:], rstd[:], op=ALU.mult)
                nc.scalar.activation(hb[:], y1[:], AF.Silu, bias=nmr[:],
                                     scale=rstd[:])
            else:
                nc.vector.tensor_scalar(y1[:], y1[:], mv[:, 0:1], rstd[:],
                                        op0=ALU.subtract, op1=ALU.mult)
                nc.vector.tensor_mul(y1[:], y1[:], g_t[:])
                nc.vector.tensor_add(y1[:], y1[:], be_t[:])
                nc.scalar.activation(hb[:], y1[:], AF.Silu)
            hb_tiles[g] = hb

        def emit_LN(g):
            emit_LN_a(g)
            emit_LN_b(g)

        def emit_evac(k):
            ps = y2ps.pop(k)
            o = opool.tile([128, D2], F32, tag="o", name=f"o_{k}")
            nc.vector.tensor_add(o[:], ps[:], b2_t[:])
            nc.sync.dma_start(out=out_d[k * 128:(k + 1) * 128, :], in_=o[:])

        def run_hooks(hooks, i):
            for fn in hooks.get(i, ()):
                fn()

        def emit_A(g_list, hooks):
            """Layer-1 sweep over tiles in g_list."""
            for gi, g in enumerate(g_list):
                pool, tg = ((ps_b, "b") if (len(g_list) > 1 and gi == 2)
                            else (ps_y1, "y1"))
                y1ps[g] = [pool.tile([128, 512], F32, tag=tg,
                                     name=f"y1ps_{g}_{h}") for h in range(2)]
            chebs = {g: {0: cheb_pre.pop(("A", g, 0))} for g in g_list}
            for i in range(8):
                run_hooks(hooks, i)
                p = i // 2
                if i % 2 == 0 and p + 1 < 4:
                    for g in g_list:
                        chebs[g][p + 1] = fill(None, "A", g, p + 1)
                for g in g_list:
                    for d in range(6):
                        st = chebs[g][p][:, d, i % 2, :]
                        for h in range(2):
                            nc.tensor.matmul(
                                y1ps[g][h][:], st,
                                w1_sb[:, i, d, h * 512:(h + 1) * 512],
                                start=(i == 0 and d == 0),
                                stop=(i == 7 and d == 5))
            run_hooks(hooks, 8)

        def emit_B_split(k, hooks):
            """Final-tile layer-2 sweep, split 384+128 across two PSUM banks
            so the wide part's bias+store overlaps the narrow part's matmuls
            (separate banks: no PSUM collision) and only a 64KB store chain
            remains after the very last matmul.  384-wide first: its
            1.0us/i-block consumption stays behind fill production."""
            chebs = {0: cheb_pre.pop(("B", k, 0)), 1: cheb_pre.pop(("B", k, 1))}
            for pn in (2, 3):
                chebs[pn] = fill(hb_tiles[k], "B", k, pn)
            o = opool.tile([128, D2], F32, tag="o", name=f"o_{k}")
            for h, sl in enumerate((slice(0, 384), slice(384, 512))):
                n = sl.stop - sl.start
                y2h = ps_b.tile([128, n], F32, tag="b", name=f"y2ps_{k}_{h}")
                for i in range(8):
                    if h == 0:
                        run_hooks(hooks, i)
                    for d in range(6):
                        nc.tensor.matmul(y2h[:], chebs[i // 2][:, d, i % 2, :],
                                         w2_sb[:, i, d, sl],
                                         start=(i == 0 and d == 0),
                                         stop=(i == 7 and d == 5))
                nc.vector.tensor_add(o[:, sl], y2h[:], b2_t[:, sl])
                nc.sync.dma_start(out=out_d[k * 128:(k + 1) * 128, sl],
                                  in_=o[:, sl])
            hb_tiles.pop(k)

        def emit_B(k, hooks):
            """Layer-2 sweep for tile k (input hb_tiles[k])."""
            y2 = ps_b.tile([128, 512], F32, tag="b", name=f"y2ps_{k}")
            y2ps[k] = y2
            chebs = {0: cheb_pre.pop(("B", k, 0))}
            for i in range(8):
                run_hooks(hooks, i)
                p = i // 2
                # deeper fill-ahead than the A sweeps: the B sweep consumes an
                # i-block every 1.28us, barely more than the ~2.8us fill chain
                # latency / 2 blocks, so produce two pairs up front.
                for pn in ((1, 2) if i == 0 else (3,) if i == 2 else ()):
                    chebs[pn] = fill(hb_tiles[k], "B", k, pn)
                for d in range(6):
                    nc.tensor.matmul(y2[:], chebs[p][:, d, i % 2, :],
                                     w2_sb[:, i, d, :],
                                     start=(i == 0 and d == 0),
                                     stop=(i == 7 and d == 5))
            hb_tiles.pop(k, None)
            run_hooks(hooks, 8)

        # ------------------------------------------------------------------
        # startup: x tiles (gpsimd queue) + their tanh first, then weights
        # (sync queue, one 1.5MB transfer per i-block, i=0 split per-d so the
        # first matmuls aren't gated on a full block), bias vectors last.
        dma_xt(0, [0, 1])

        w1_sb = wpool.tile([128, 8, 6, D1], BF16, name="w1_sb")
        w2_sb = wpool.tile([128, 8, 6, D2], BF16, name="w2_sb")
        for i in range(3):
            for d in range(6):
                src = w1_d[i, :, d, :]
                if i == 0 and d == 0:
                    for q in range(2):
                        nc.sync.dma_start(
                            out=w1_sb[:, 0, 0, q * 512:(q + 1) * 512],
                            in_=src[:, q * 512:(q + 1) * 512])
                else:
                    nc.sync.dma_start(out=w1_sb[:, i, d, :], in_=src)
            if i == 0:
                dma_xt(0, [2, 3, 4, 5, 6, 7])
                dma_xt(1)
        for i in range(3, 8):
            nc.sync.dma_start(out=w1_sb[:, i], in_=w1_d[i])
        for i in range(8):
            nc.sync.dma_start(out=w2_sb[:, i], in_=w2_d[i])
        # ident is first needed by the first layer-2 prefill transpose (~80us)
        nc.sync.dma_start(out=ident[:], in_=ident_dram.ap())

        b1_t = _bcast_row(nc, consts, b1_d, D1, "b1_t")
        b2_t = _bcast_row(nc, consts, b2_d, D2, "b2_t")
        g_t = (None if trivial else _bcast_row(nc, consts, g_d, D1, "g_t", BF16))
        be_t = (None if trivial else _bcast_row(nc, consts, be_d, D1, "be_t",
                                                BF16))

        for g in range(3):
            cheb_pre[("A", g, 0)] = fill(None, "A", g, 0)

        # ------------------------------------------------------------------
        # segment schedule: S0=A{0,1}, A2, A3, B0, A4, B1, ..., A15, B12..B15
        def setd(key, tag, g, p, src_map):
            cheb_pre[key] = fill(None if src_map is None else src_map[g],
                                 tag, g, p)

        emit_A([0, 1, 2], hooks={
            4: [lambda: dma_xt(2)],
            5: [lambda: setd(("A", 3, 0), "A", 3, 0, None)],
            8: [lambda: emit_LN(0)],
        })

        def A_hooks(g):
            h = {}
            add = lambda i, fn: h.setdefault(i, []).append(fn)
            if g == 3:
                add(1, lambda: emit_LN_a(1))
                add(2, lambda: emit_LN_b(1))
                add(3, lambda: setd(("B", 0, 0), "B", 0, 0, hb_tiles))
                add(4, lambda: emit_LN_a(2))
                add(5, lambda: emit_LN_b(2))
            else:
                add(1, lambda: emit_LN_a(g - 1))
                # prefill for B_{g-3} at i==2: ahead of this segment's p2/p3
                # fills and LN_b in the ACT/DVE queues, so its ~2.8us serial
                # chain lands well before the B sweep starts
                add(2, lambda: setd(("B", g - 3, 0), "B", g - 3, 0, hb_tiles))
                add(3, lambda: emit_LN_b(g - 1))
            if g >= 4:
                add(2, lambda: emit_evac(g - 4))
            if g in (3, 5, 7, 9, 11):
                add(4, lambda: dma_xt((g + 3) // 2))
            return h

        def B_hooks(k):
            h = {}
            add = lambda i, fn: h.setdefault(i, []).append(fn)
            if k + 4 <= 15:
                # next segment is A_{k+4}
                add(4, lambda: setd(("A", k + 4, 0), "A", k + 4, 0, None))
            if k == 12:
                add(3, lambda: setd(("B", 13, 0), "B", 13, 0, hb_tiles))
            if k == 13:
                # LN(15) here (not at B12): writing hb(15) recycles hb(12)'s
                # slot, whose readers (B12's fills) must all be emitted first.
                add(1, lambda: emit_LN_a(15))
                add(3, lambda: emit_LN_b(15))
            if k in (13, 14):
                add(4, lambda: setd(("B", k + 1, 0), "B", k + 1, 0, hb_tiles))
            if k == 14:
                add(5, lambda: setd(("B", 15, 1), "B", 15, 1, hb_tiles))
            if k >= 13:
                add(2, lambda: emit_evac(k - 1))
            return h

        emit_A([3], A_hooks(3))
        for k in range(12):
            emit_B(k, B_hooks(k))
            emit_A([k + 4], A_hooks(k + 4))
        for k in range(12, 15):
            emit_B(k, B_hooks(k))
        emit_B_split(15, B_hooks(15))


_PROGRAMS = {}


def _get_program(trivial_affine: bool):
    key = trivial_affine
    if key in _PROGRAMS:
        return _PROGRAMS[key]
    nc = bacc.Bacc("TRN2", target_bir_lowering=False, debug=False,
                   num_devices=N_CORES)
    x_d = nc.dram_tensor("x_in", [8, 128, BC], BF16, kind="ExternalInput").ap()
    w1_d = nc.dram_tensor("w1", [8, 128, 6, D1], BF16, kind="ExternalInput").ap()
    w2_d = nc.dram_tensor("w2", [8, 128, 6, D2], BF16, kind="ExternalInput").ap()
    b1_d = nc.dram_tensor("b1e", [D1], F32, kind="ExternalInput").ap()
    b2_d = nc.dram_tensor("b2e", [D2], F32, kind="ExternalInput").ap()
    if trivial_affine:
        g_d = be_d = None
    else:
        g_d = nc.dram_tensor("gam", [D1], BF16, kind="ExternalInput").ap()
        be_d = nc.dram_tensor("bet", [D1], BF16, kind="ExternalInput").ap()
    out_d = nc.dram_tensor("out", [BC, D2], F32, kind="ExternalOutput").ap()

    with tile.TileContext(nc) as tc:
        _kernel_body(tc, out_d, x_d, w1_d, w2_d, b1_d, b2_d, g_d, be_d)
    nc.compile()
    _PROGRAMS[key] = nc
    return nc


def _prep_inputs(x, coeff1, base_w1, bias1, ln_gamma, ln_beta, coeff2,
                 base_w2, bias2):
    # feature-major (transposed) per-core x: [i_block, feat_in_block, batch]
    x = np.asarray(x, np.float32).astype(ml_dtypes.bfloat16)
    coeff1 = np.asarray(coeff1, np.float32)
    coeff2 = np.asarray(coeff2, np.float32)

    # layout: [i_block, row_in_block, d, out] so each i-block is one
    # contiguous 1.5MB DMA transfer.
    w1 = np.empty((8, 128, 6, D1), ml_dtypes.bfloat16)
    w1[:, :, 0, :] = np.asarray(base_w1, np.float32).T.reshape(8, 128, D1)
    for d in range(1, 6):
        w1[:, :, d, :] = coeff1[:, :, d].T.reshape(8, 128, D1)
    w2 = np.empty((8, 128, 6, D2), ml_dtypes.bfloat16)
    w2[:, :, 0, :] = np.asarray(base_w2, np.float32).T.reshape(8, 128, D2)
    for d in range(1, 6):
        w2[:, :, d, :] = coeff2[:, :, d].T.reshape(8, 128, D2)
    b1e = (np.asarray(bias1, np.float32)
           + coeff1[:, :, 0].sum(axis=1)).astype(np.float32)
    b2e = (np.asarray(bias2, np.float32)
           + coeff2[:, :, 0].sum(axis=1)).astype(np.float32)

    g = np.asarray(ln_gamma, np.float32)
    be = np.asarray(ln_beta, np.float32)
    trivial = bool(np.all(g == 1.0) and np.all(be == 0.0))

    shared = {"w1": w1, "w2": w2, "b1e": b1e, "b2e": b2e}
    if not trivial:
        shared["gam"] = g.astype(ml_dtypes.bfloat16)
        shared["bet"] = be.astype(ml_dtypes.bfloat16)
    in_maps = []
    for cid in range(N_CORES):
        m = dict(shared)
        xc = x[cid * BC:(cid + 1) * BC]
        m["x_in"] = np.ascontiguousarray(xc.T.reshape(8, 128, BC))
        in_maps.append(m)
    return trivial, in_maps


def kernel_run(trace=False, **inputs):
    trivial, in_maps = _prep_inputs(**inputs)
    nc = _get_program(trivial)
    res = run_bass_kernel_spmd(nc, in_maps, core_ids=list(range(N_CORES)),
                               trace=trace)
    out = np.concatenate([r["out"] for r in res.results], axis=0)
    return out, res


def kernel(**inputs):
    out, _ = kernel_run(trace=False, **inputs)
    return out
